# revision 1
# baseline (speedup 1.0000x reference)
"""Trainium2 Bass kernel for nn_DecouplingFlowLayer.

Computes, for x [B=4, S=128, N=512, F=362] fp32:
  X_l_proj = (x with feature0 := Haar-lowpass)  @ Wg^T + Wg_b   -> [B,S,N,64]
  X_h_proj = (x with feature0 := Haar-highpass) @ Wh^T + Wh_b   -> [B,S,N,64]

Strategy (per NeuronCore, data-parallel over B*S across 8 cores):
  - One "tile" = one (b,s) slice = 512 tokens x 362 features (contiguous in HBM).
  - DMA natural layout [128 tok, 4 chunks, 364 cols]; the 2 extra columns get
    the Haar pair terms (written by DVE from the paired tile's feature-0):
      col 362 = x0_pair - x0_self   (weight row 362 = 0.5*Wg[:,0] | 0)
      col 363 = x0_self + x0_pair   (weight row 363 = 0 | -0.5*Wh[:,0])
    which makes the single GEMM over all 364 "features" produce both the
    original projection and the Haar feature-0 replacement as a rank-1 update.
  - TensorE transposes 128x128 blocks -> [f, tok] tiles, one combined GEMM
    (K=364 in 3 chunks) with W_aug [364, 128] (cols 0:64 = Wg^T, 64:128 = Wh^T)
    -> out [128 d, 512 tok] in PSUM.
  - ScalarE adds the biases while copying PSUM->SBUF, TensorE transposes the
    output back to [tok, d], and DMA stores contiguous [512,64] slabs per
    output tensor.
"""

import numpy as np

import concourse.bass as bass
import concourse.mybir as mybir
from concourse.bass_utils import run_bass_kernel_spmd
from concourse.tile import TileContext

F32 = mybir.dt.float32

# dtypes used on the PE array (bitcast views of the same fp32 bits)
# float32r streams 1 row/cycle for moving free dim >=256 (vs 4 cyc/row for
# plain fp32) and is validated to be fp32-exact by test.py's rel-err check.
MM_DT = mybir.dt.float32r     # matmul operand dtype (float32 | float32r)
TR_DT = mybir.dt.float32      # transpose operand dtype

N_CORES = 8
B, S, N, F, D = 4, 128, 512, 362, 64
BS = B * S                     # 512 (b,s) slices
TPC = BS // N_CORES            # 64 slices (tiles) per core
FW = F + 2                     # 364: features + haar-delta + haar-sum
FBLK = [(0, 128), (128, 128), (256, FW - 256)]  # K blocks (last = 108)


def _patch_drain():
    """walrus (TRN2) can encode only one sync-wait per instruction for several
    instruction formats (Matmult/S3_LW, SP CTRL drain, ...). Tile's scheduler
    happily attaches 2+ waits. Hoist excess waits onto standalone
    InstEventSemaphore instructions on the same engine (identical sequencer
    stall semantics), keeping one wait on the original instruction."""
    import concourse.tile as tile_mod
    from concourse.vector_clock import ScopedClock

    if getattr(tile_mod.TileContext, "_drain_split_patch", False):
        return

    orig_cal = tile_mod.TileContext._commit_and_lower

    def _commit_and_lower(self, inst, original_block, old_bb_map, bb_to_exit_bb):
        si = getattr(inst, "sync_info", None)
        waits = list(si.on_wait) if (si and si.on_wait) else []
        if (
            len(waits) > 1
            and isinstance(inst, mybir.Instruction)
            and inst.engine != mybir.EngineType.Unassigned
            and not type(inst).__name__.startswith("BassTile")
        ):
            for w in waits[:-1]:
                ev = mybir.InstEventSemaphore(
                    name=f"EVW-{self.nc.next_id()}",
                    ins=[],
                    outs=[],
                    sync_info=mybir.SyncInfo(on_wait=[w], on_update=[]),
                )
                ev.engine = inst.engine
                orig_cal(self, ev, original_block, old_bb_map, bb_to_exit_bb)
            inst.sync_info = mybir.SyncInfo(
                on_wait=[waits[-1]], on_update=list(si.on_update or [])
            )
        return orig_cal(self, inst, original_block, old_bb_map, bb_to_exit_bb)

    tile_mod.TileContext._commit_and_lower = _commit_and_lower

    def _drain_and_barrier(self, tick_clock, wait_clock):
        nc = self.nc
        drain_inst = nc.sync.drain()
        wait_clock.add_sem_waits(
            drain_inst.ins, ScopedClock({None: tick_clock.global_clock})
        )
        si = drain_inst.ins.sync_info
        waits = list(si.on_wait or [])
        if len(waits) > 1:
            drain_inst.ins.sync_info = mybir.SyncInfo(
                on_wait=waits[:1], on_update=list(si.on_update or [])
            )
            for i in range(1, len(waits)):
                extra = nc.sync.drain()
                extra.ins.sync_info = mybir.SyncInfo(
                    on_wait=waits[i : i + 1], on_update=[]
                )
        nc.all_engine_barrier()
        assert self.sems is not None
        popped = nc._tile_sem_poison_stack.pop()
        assert popped is self._sem_poison
        nc.clear_and_free_semaphores(list(self.sems.allocated().values()))
        nc.all_engine_barrier()

    tile_mod.TileContext._drain_and_barrier = _drain_and_barrier
    tile_mod.TileContext._drain_split_patch = True


def _patch_birsim_off():
    """The walrus BIR-simulation pass re-executes every instruction on host
    and dominates compile time (~19 min for this kernel vs <1 s without).
    It is a validation-only pass; disable it for our compiles."""
    import concourse.bass_utils as bu

    if getattr(bu, "_birsim_off_patch", False):
        return
    orig = bu.bir_verify_and_optimise

    def patched(tmpdir, inp="bir.json", outp="file.neff", arch=None, *, dve_root=None):
        real_run = bu.run_command

        def run_hook(cmd, **kw):
            cmd = [
                "--enable-birsim=false" if c == "--enable-birsim=true" else c
                for c in cmd
            ]
            return real_run(cmd, **kw)

        bu.run_command = run_hook
        try:
            return orig(tmpdir, inp, outp, arch, dve_root=dve_root)
        finally:
            bu.run_command = real_run

    bu.bir_verify_and_optimise = patched
    bu._birsim_off_patch = True


def _build_nc():
    _patch_drain()
    _patch_birsim_off()
    nc = bass.Bass("TRN2", target_bir_lowering=False, debug=False)

    x_d = nc.declare_dram_parameter("x", [TPC, 4, 128, F], F32, isOutput=False)
    w_d = nc.declare_dram_parameter("w", [FW, 128], F32, isOutput=False)
    bias_d = nc.declare_dram_parameter("bias", [128, 1], F32, isOutput=False)
    id_d = nc.declare_dram_parameter("ident", [128, 128], F32, isOutput=False)
    o_d = nc.declare_dram_parameter("out", [2, TPC, 512, 64], F32, isOutput=True)

    with TileContext(nc) as tc:
        with (
            tc.tile_pool(name="const", bufs=1) as cpool,
            tc.tile_pool(name="nat", bufs=3) as natp,
            tc.tile_pool(name="xt", bufs=12) as xtp,
            tc.tile_pool(name="osb", bufs=4) as osbp,
            tc.tile_pool(name="stg", bufs=2) as stgp,
            tc.tile_pool(name="pxt", bufs=4, space="PSUM") as pxtp,
            tc.tile_pool(name="pmm", bufs=2, space="PSUM") as pmmp,
            tc.tile_pool(name="pot", bufs=2, space="PSUM") as potp,
        ):
            ws = []
            for k, (f0, fk) in enumerate(FBLK):
                wf = cpool.tile([128, 128], F32, tag=f"wf{k}", name=f"wf{k}")
                nc.sync.dma_start(out=wf[0:fk, :], in_=w_d[f0 : f0 + fk, :])
                wk = cpool.tile([128, 128], MM_DT, tag=f"w{k}", name=f"w{k}")
                nc.vector.tensor_copy(wk[0:fk, :], wf[0:fk, :])
                ws.append(wk)
            ident = cpool.tile([128, 128], F32, tag="ident", name="ident")
            nc.sync.dma_start(out=ident[:, :], in_=id_d[:, :])
            bias = cpool.tile([128, 1], F32, tag="bias", name="bias")
            nc.sync.dma_start(out=bias[:, :], in_=bias_d[:, :])

            for qd in range(TPC // 4):
                # one ~3 MB load per quad of 4 (b,s) slices (2 Haar pairs)
                nat = natp.tile([128, 4, 4, FW], F32, tag="nat", name="nat")
                nc.sync.dma_start(
                    out=nat[:, :, :, 0:F],
                    in_=x_d[4 * qd : 4 * qd + 4].rearrange("t c p f -> p t c f"),
                )
                for pp in range(2):
                    e, o = 2 * pp, 2 * pp + 1
                    x0e, x0o = nat[:, e, :, 0], nat[:, o, :, 0]
                    # col 362: (pair - self); col 363: (self + pair)
                    nc.vector.tensor_sub(nat[:, e, :, F], x0o, x0e)
                    nc.vector.tensor_sub(nat[:, o, :, F], x0e, x0o)
                    nc.vector.tensor_add(nat[:, e, :, F + 1], x0e, x0o)
                    nc.vector.tensor_copy(nat[:, o, :, F + 1], nat[:, e, :, F + 1])

                stg = stgp.tile([128, 4, 2, 4, 64], F32, tag="stg", name="stg")
                for ti in range(4):
                    # ---- transpose x into [f, tok] tiles ----
                    xts = []
                    for k, (f0, fk) in enumerate(FBLK):
                        pxt = pxtp.tile([128, 512], F32, tag="pxt", name="pxt")
                        for c in range(4):
                            nc.tensor.transpose(
                                pxt[0:fk, c * 128 : (c + 1) * 128],
                                nat[:, ti, c, f0 : f0 + fk],
                                ident[:, :],
                            )
                        xt = xtp.tile([128, 512], MM_DT, tag="xt", name="xt")
                        nc.vector.tensor_copy(xt[0:fk, :], pxt[0:fk, :])
                        xts.append(xt)
                    # ---- GEMM: out[d, tok] += W_aug[f, d]^T @ xT[f, tok] ----
                    pmm = pmmp.tile([128, 512], F32, tag="pmm", name="pmm")
                    for k, (f0, fk) in enumerate(FBLK):
                        nc.tensor.matmul(
                            pmm[:, :],
                            ws[k][0:fk, :],
                            xts[k][0:fk, :],
                            start=(k == 0),
                            stop=(k == len(FBLK) - 1),
                        )
                    # ---- bias add + PSUM->SBUF ----
                    osb = osbp.tile([128, 512], F32, tag="osb", name="osb")
                    nc.scalar.add(osb[:, :], pmm[:, :], bias[:, :])
                    # ---- transpose back to [tok, d] ----
                    pot = potp.tile([128, 512], F32, tag="pot", name="pot")
                    osb_q = osb.rearrange("p (t q) -> p q t", q=4)
                    for t in range(4):
                        nc.tensor.transpose(
                            pot[:, t * 128 : (t + 1) * 128],
                            osb_q[:, t, :],
                            ident[:, :],
                        )
                    nc.scalar.copy(
                        stg[:, ti],
                        pot.rearrange("p (q lh d) -> p lh q d", q=4, lh=2),
                    )
                # one ~0.5 MB store per (quad, output tensor) on the ACT ring
                for lh in range(2):
                    nc.scalar.dma_start(
                        out=o_d[lh, 4 * qd : 4 * qd + 4].rearrange(
                            "t (p q) d -> p t q d", q=4
                        ),
                        in_=stg[:, :, lh],
                    )
    return nc


_NC = None


def kernel(x, Wg_w, Wg_b, Wh_w, Wh_b):
    global _NC
    if _NC is None:
        _NC = _build_nc()

    x = np.ascontiguousarray(np.asarray(x, dtype=np.float32))
    Wg_w = np.asarray(Wg_w, dtype=np.float32)
    Wg_b = np.asarray(Wg_b, dtype=np.float32)
    Wh_w = np.asarray(Wh_w, dtype=np.float32)
    Wh_b = np.asarray(Wh_b, dtype=np.float32)

    waug = np.zeros((FW, 128), dtype=np.float32)
    waug[:F, :64] = Wg_w.T
    waug[:F, 64:] = Wh_w.T
    waug[F, :64] = 0.5 * Wg_w[:, 0]
    waug[F + 1, 64:] = -0.5 * Wh_w[:, 0]
    biasv = np.concatenate([Wg_b, Wh_b]).astype(np.float32).reshape(128, 1)
    ident = np.eye(128, dtype=np.float32)

    xf = x.reshape(BS, N, F)
    in_maps = []
    for i in range(N_CORES):
        shard = xf[i * TPC : (i + 1) * TPC].reshape(TPC, 4, 128, F)
        in_maps.append(
            {"x": shard, "w": waug, "bias": biasv, "ident": ident}
        )

    res = run_bass_kernel_spmd(_NC, in_maps, list(range(N_CORES)))
    out_l = np.concatenate(
        [res.results[i]["out"][0] for i in range(N_CORES)], axis=0
    ).reshape(B, S, N, D)
    out_h = np.concatenate(
        [res.results[i]["out"][1] for i in range(N_CORES)], axis=0
    ).reshape(B, S, N, D)
    return (out_l, out_h)

